# revision 1
# baseline (speedup 1.0000x reference)
"""Multi-head attention TRN2 kernel.

Problem: B=4, T=2048, D=1024, H=16, Hd=64.
  qkv = x @ w_qkv + b_qkv ; attention per head ; out = attn @ w_out + b_out

Sharding over 8 cores: core = (b, g) with b in 0..3 (batch), g in 0..1
(head group of 8 heads).  Each core computes a partial output
x[b]-rows x full-D; host sums the two head-group partials per batch.

Host-side folds (all exact, free):
  * 1/sqrt(Hd)=1/8 folded into w_q, b_q.
  * b_k dropped: adds a per-query constant to scores -> softmax invariant.
  * b_v folded into the output bias: softmax rows sum to 1, so
    attn(v + b_v) = attn(v) + b_v; b_eff = b_v @ w_out + b_out added on host.

Per-core dataflow (all activations kept "transposed", feature-on-partition):
  xT[d,t] (bf16)  --MM w_qkv-->  Q^T,K^T [hd, t]   V [t, hd] (natural)
  S^T[k,q] = K^T.T-chunks @ Q^T  (contraction over d=64, per head)
  P^T = exp(S^T)  via ScalarE ACT, PSUM->SBUF bf16
  AV: lhsT = [V | ones] (128 cols) -> psum rows 0:64 = attnU^T, rows 64:128 =
      softmax denominators (broadcast for free)
  attn^T = attnU^T * reciprocal(denominator)   (VectorE)
  out[t, :] = attn^T.T-chunks @ w_out-slice  -> DRAM partial
"""

import os
import sys

import numpy as np
import ml_dtypes

for _p in ("/opt/trn_rl_repo",):
    if _p not in sys.path:
        sys.path.append(_p)

import concourse.bass as bass
import concourse.tile as tile
from concourse import bacc, mybir
from concourse.bass import ts

dt = mybir.dt
AF = mybir.ActivationFunctionType
BF16 = ml_dtypes.bfloat16

T = 2048
D = 1024
HPC = 8          # heads per core
HD = 64
DCH = D // 128   # 8 contraction chunks for qkv proj
NPAIR = 4        # head pairs per core
KCH = T // 128   # 16 key chunks


def build_nc(debug: bool = False):
    nc = bacc.Bacc("TRN2", target_bir_lowering=False, debug=debug)

    xT_d = nc.dram_tensor("xT", [128, DCH, T], dt.bfloat16, kind="ExternalInput")
    wqk_d = nc.dram_tensor("wqk", [128, DCH, 1024], dt.bfloat16, kind="ExternalInput")
    wv_d = nc.dram_tensor("wv", [128, DCH, 512], dt.bfloat16, kind="ExternalInput")
    wo_d = nc.dram_tensor("wo", [128, NPAIR, 1024], dt.bfloat16, kind="ExternalInput")
    bq_d = nc.dram_tensor("bq", [128, NPAIR], dt.float32, kind="ExternalInput")
    out_d = nc.dram_tensor("out", [T, 1024], dt.float32, kind="ExternalOutput")

    with tile.TileContext(nc) as tc:
        with (
            tc.tile_pool(name="const", bufs=1) as cpool,
            tc.tile_pool(name="xt", bufs=2) as xpool,
            tc.tile_pool(name="qk", bufs=1) as qkpool,
            tc.tile_pool(name="vaug", bufs=1) as vpool,
            tc.tile_pool(name="pt", bufs=3) as ptpool,
            tc.tile_pool(name="attn", bufs=1) as apool,
            tc.tile_pool(name="recip", bufs=2) as rpool,
            tc.tile_pool(name="outst", bufs=3) as opool,
            tc.tile_pool(name="ps_s", bufs=2, space="PSUM") as ps_s,
            tc.tile_pool(name="ps_av", bufs=2, space="PSUM") as ps_av,
        ):
            # ---- constants ----
            wqk = cpool.tile([128, DCH, 1024], dt.bfloat16, tag="wqk")
            nc.sync.dma_start(wqk[:], wqk_d[:])
            wv = cpool.tile([128, DCH, 512], dt.bfloat16, tag="wv")
            nc.sync.dma_start(wv[:], wv_d[:])
            wo = cpool.tile([128, NPAIR, 1024], dt.bfloat16, tag="wo")
            nc.sync.dma_start(wo[:], wo_d[:])
            bq = cpool.tile([128, NPAIR], dt.float32, tag="bq")
            nc.sync.dma_start(bq[:], bq_d[:])

            # Q^T / K^T: [64*(h%2)+d, pair, t]
            qt = qkpool.tile([128, NPAIR, T], dt.bfloat16, tag="qt")
            kt = qkpool.tile([128, NPAIR, T], dt.bfloat16, tag="kt")
            # V augmented: [t%128, kchunk, head, 64 vals + 64 ones]
            vva = vpool.tile([128, KCH, HPC, 128], dt.bfloat16, tag="vaug")
            nc.any.memset(vva[:, :, :, 64:128], 1.0)
            # attn^T accumulator: [64*(h%2)+d, pair, t]
            attn = apool.tile([128, NPAIR, T], dt.bfloat16, tag="attn")

            # ---- phase 1: qkv projection ----
            for tc5 in range(T // 512):
                xt = xpool.tile([128, DCH, 512], dt.bfloat16, tag="xt")
                nc.sync.dma_start(xt[:], xT_d[:, :, ts(tc5, 512)])
                # Q^T (cc 0..3) and K^T (cc 4..7), out = [col, t]
                for cc in range(8):
                    ps = ps_s.tile([128, 1024], dt.float32, tag="ps_s")
                    for dc in range(DCH):
                        nc.tensor.matmul(
                            ps[:, 0:512],
                            wqk[:, dc, ts(cc, 128)],
                            xt[:, dc, :],
                            start=(dc == 0),
                            stop=(dc == DCH - 1),
                        )
                    if cc < 4:
                        nc.vector.tensor_scalar_add(
                            qt[:, cc, ts(tc5, 512)], ps[:, 0:512], bq[:, cc : cc + 1]
                        )
                    else:
                        nc.vector.tensor_copy(
                            kt[:, cc - 4, ts(tc5, 512)], ps[:, 0:512]
                        )
                # V natural, out = [t, head*64+d]
                for t8 in range(4):
                    kc = tc5 * 4 + t8
                    ps = ps_s.tile([128, 1024], dt.float32, tag="ps_s")
                    for dc in range(DCH):
                        nc.tensor.matmul(
                            ps[:, 0:512],
                            xt[:, dc, ts(t8, 128)],
                            wv[:, dc, :],
                            start=(dc == 0),
                            stop=(dc == DCH - 1),
                        )
                    nc.vector.tensor_copy(
                        vva[:, kc, :, 0:64],
                        ps[:, 0:512].rearrange("p (h d) -> p h d", h=HPC),
                    )

            # ---- phase 2: attention per head ----
            for h in range(HPC):
                pair, base = h // 2, 64 * (h % 2)
                for qh in range(2):  # halves of q (1024 each)
                    av = ps_av.tile([128, 1024], dt.float32, tag="ps_av")
                    for kc in range(KCH):
                        ps = ps_s.tile([128, 1024], dt.float32, tag="ps_s")
                        for qq in range(2):
                            nc.tensor.matmul(
                                ps[:, ts(qq, 512)],
                                kt[base : base + 64, pair, ts(kc, 128)],
                                qt[base : base + 64, pair,
                                   qh * 1024 + qq * 512 : qh * 1024 + (qq + 1) * 512],
                                start=True,
                                stop=True,
                            )
                        pt = ptpool.tile([128, 1024], dt.bfloat16, tag="pt")
                        nc.scalar.activation(pt[:], ps[:], AF.Exp)
                        for qq in range(2):
                            nc.tensor.matmul(
                                av[:, ts(qq, 512)],
                                vva[:, kc, h, :],
                                pt[:, ts(qq, 512)],
                                start=(kc == 0),
                                stop=(kc == KCH - 1),
                            )
                    rc = rpool.tile([64, 1024], dt.float32, tag="recip")
                    nc.vector.reciprocal(rc[:], av[64:128, :])
                    nc.vector.tensor_tensor(
                        attn[base : base + 64, pair, qh * 1024 : (qh + 1) * 1024],
                        av[0:64, :],
                        rc[:],
                        mybir.AluOpType.mult,
                    )

            # ---- phase 3: output projection (partial; bias added on host) ----
            for t8 in range(T // 128):
                ps = ps_s.tile([128, 1024], dt.float32, tag="ps_s")
                for ncol in range(2):
                    for hc in range(NPAIR):
                        nc.tensor.matmul(
                            ps[:, ts(ncol, 512)],
                            attn[:, hc, ts(t8, 128)],
                            wo[:, hc, ts(ncol, 512)],
                            start=(hc == 0),
                            stop=(hc == NPAIR - 1),
                        )
                ot = opool.tile([128, 1024], dt.float32, tag="outst")
                nc.any.tensor_copy(ot[:], ps[:])
                nc.sync.dma_start(out_d[ts(t8, 128), :], ot[:])

    nc.compile()
    return nc


def prep_in_maps(x, w_qkv, b_qkv, w_out, b_out):
    """Host-side shard prep. Returns (in_maps for cores 0..7, b_eff)."""
    x = np.asarray(x, dtype=np.float32)
    w_qkv = np.asarray(w_qkv, dtype=np.float32)
    b_qkv = np.asarray(b_qkv, dtype=np.float32)
    w_out = np.asarray(w_out, dtype=np.float32)
    b_out = np.asarray(b_out, dtype=np.float32)

    wq, wk, wv_full = w_qkv[:, 0:D], w_qkv[:, D : 2 * D], w_qkv[:, 2 * D : 3 * D]
    bq_full = b_qkv[0:D]
    bv = b_qkv[2 * D : 3 * D]
    b_eff = (bv @ w_out + b_out).astype(np.float32)

    in_maps = []
    for core in range(8):
        b, g = core // 2, core % 2
        gs = slice(g * 512, (g + 1) * 512)
        wqk_c = np.concatenate([wq[:, gs] * 0.125, wk[:, gs]], axis=1)  # [1024,1024]
        wqk_c = wqk_c.reshape(DCH, 128, 1024).transpose(1, 0, 2)
        wv_c = wv_full[:, gs].reshape(DCH, 128, 512).transpose(1, 0, 2)
        wo_c = w_out[gs, :].reshape(NPAIR, 128, 1024).transpose(1, 0, 2)
        bq_c = (bq_full[gs] * 0.125).reshape(NPAIR, 128).T
        xT_c = x[b].T.reshape(DCH, 128, T).transpose(1, 0, 2)
        in_maps.append(
            {
                "xT": np.ascontiguousarray(xT_c).astype(BF16),
                "wqk": np.ascontiguousarray(wqk_c).astype(BF16),
                "wv": np.ascontiguousarray(wv_c).astype(BF16),
                "wo": np.ascontiguousarray(wo_c).astype(BF16),
                "bq": np.ascontiguousarray(bq_c).astype(np.float32),
            }
        )
    return in_maps, b_eff


def gather_output(core_outs, b_eff):
    """core_outs: list of 8 [T, D] partials (core order b*2+g)."""
    out = np.empty((4, T, D), dtype=np.float32)
    for b in range(4):
        out[b] = core_outs[2 * b] + core_outs[2 * b + 1] + b_eff
    return out


_NC_CACHE = None


def _get_nc():
    global _NC_CACHE
    if _NC_CACHE is None:
        _NC_CACHE = build_nc(debug=False)
    return _NC_CACHE


def kernel(x, w_qkv, b_qkv, w_out, b_out):
    from concourse.bass_utils import run_bass_kernel_spmd

    in_maps, b_eff = prep_in_maps(x, w_qkv, b_qkv, w_out, b_out)
    nc = _get_nc()
    res = run_bass_kernel_spmd(nc, in_maps, core_ids=list(range(8)))
    return gather_output([r["out"] for r in res.results], b_eff)


# revision 22
# speedup vs baseline: 245.2978x; 245.2978x over previous
"""Multi-head attention TRN2 kernel.

Problem: B=4, T=2048, D=1024, H=16, Hd=64.
  qkv = x @ w_qkv + b_qkv ; attention per head ; out = attn @ w_out + b_out

Sharding over 8 cores: core = (b, g) with b in 0..3 (batch), g in 0..1
(head group of 8 heads).  Each core computes a partial output
x[b]-rows x full-D; host sums the two head-group partials per batch.

Host-side folds (all exact, free):
  * 1/sqrt(Hd)=1/8 folded into w_q, b_q.
  * b_k dropped: adds a per-query constant to scores -> softmax invariant.
  * b_v folded into the output bias: softmax rows sum to 1, so
    attn(v + b_v) = attn(v) + b_v; b_eff = b_v @ w_out + b_out added on host.

Per-core dataflow (all activations kept "transposed", feature-on-partition):
  xT[d,t] (bf16)  --MM w_qkv-->  Q^T,K^T [hd, t]   V [t, hd] (natural)
  S^T[k,q] = K^T.T-chunks @ Q^T  (contraction over d=64, per head)
  P^T = exp(S^T)  via ScalarE ACT, PSUM->SBUF bf16
  AV: lhsT = [V | ones] (128 cols) -> psum rows 0:64 = attnU^T, rows 64:128 =
      softmax denominators (broadcast for free)
  attn^T = attnU^T * reciprocal(denominator)   (VectorE)
  out[t, :] = attn^T.T-chunks @ w_out-slice  -> DRAM partial
"""

import os
import sys

import numpy as np
import ml_dtypes

for _p in ("/opt/trn_rl_repo",):
    if _p not in sys.path:
        sys.path.append(_p)

import concourse.bass as bass
import concourse.tile as tile
from concourse import bacc, mybir
from concourse.bass import ts

dt = mybir.dt
AF = mybir.ActivationFunctionType
BF16 = ml_dtypes.bfloat16

T = 2048
D = 1024
HPC = 8          # heads per core
HD = 64
DCH = D // 128   # 8 contraction chunks for qkv proj
NPAIR = 4        # head pairs per core
KCH = T // 128   # 16 key chunks


def build_nc(debug: bool = False, reps: int = 1):
    nc = bacc.Bacc("TRN2", target_bir_lowering=False, debug=debug)

    xT_d = nc.dram_tensor("xT", [128, DCH, T], dt.bfloat16, kind="ExternalInput")
    wqk_d = nc.dram_tensor("wqk", [128, DCH, 1024], dt.bfloat16, kind="ExternalInput")
    wv_d = nc.dram_tensor("wv", [128, DCH, 512], dt.bfloat16, kind="ExternalInput")
    wo_d = nc.dram_tensor("wo", [128, NPAIR, 1024], dt.bfloat16, kind="ExternalInput")
    bq_d = nc.dram_tensor("bq", [128, NPAIR], dt.float32, kind="ExternalInput")
    out_d = nc.dram_tensor("out", [T, 1024], dt.float32, kind="ExternalOutput")

    with tile.TileContext(nc) as tc:
        with (
            tc.tile_pool(name="const", bufs=1) as cpool,
            tc.tile_pool(name="xt", bufs=1) as xpool,
            tc.tile_pool(name="qk", bufs=1) as qkpool,
            tc.tile_pool(name="vaug", bufs=1) as vpool,
            tc.tile_pool(name="pt", bufs=8) as ptpool,
            tc.tile_pool(name="attn", bufs=1) as apool,
            tc.tile_pool(name="recip", bufs=4) as rpool,
            tc.tile_pool(name="outst", bufs=3) as opool,
            tc.tile_pool(name="ps_s", bufs=2, space="PSUM") as ps_s,
            tc.tile_pool(name="ps_av", bufs=2, space="PSUM") as ps_av,
            tc.tile_pool(name="ps_qk", bufs=2, space="PSUM") as ps_qk,
        ):
            # ---- constants (DMAs emitted in emit_body, ordered by first use)
            bq = cpool.tile([128, NPAIR], dt.float32, tag="bq")
            nc.sync.dma_start(bq[:], bq_d[:])
            wqk = cpool.tile([128, DCH, 1024], dt.bfloat16, tag="wqk")
            wv = cpool.tile([128, DCH, 512], dt.bfloat16, tag="wv")
            wo = cpool.tile([128, NPAIR, 1024], dt.bfloat16, tag="wo")

            # Q^T / K^T: [64*(h%2)+d, pair, t]
            qt = qkpool.tile([128, NPAIR, T], dt.bfloat16, tag="qt")
            kt = qkpool.tile([128, NPAIR, T], dt.bfloat16, tag="kt")
            # V augmented: [t%128, kchunk, head, 64 vals + 64 ones]
            vva = vpool.tile([128, KCH, HPC, 128], dt.bfloat16, tag="vaug")
            nc.any.memset(vva[:, :, :, 64:128], 1.0)
            # attn^T accumulator: [64*(h%2)+d, pair, t]
            attn = apool.tile([128, NPAIR, T], dt.bfloat16, tag="attn")

            def emit_body(first_rep):
                # resident x^T; chunk 0 first, weights right behind it so the
                # first QK burst (needs xf[:, 0, t 0:512] + wqk dc0) starts
                # after ~1.3MB of DMA instead of the full 13MB.
                xf = xpool.tile([128, DCH, T], dt.bfloat16, tag="xf")
                nc.sync.dma_start(xf[:, :, 0:512], xT_d[:, :, 0:512])
                if first_rep:
                    for dc in range(DCH):
                        nc.sync.dma_start(wqk[:, dc, :], wqk_d[:, dc, :])
                        nc.sync.dma_start(wv[:, dc, :], wv_d[:, dc, :])
                for tc5 in range(1, 4):
                    nc.sync.dma_start(
                        xf[:, :, ts(tc5, 512)], xT_d[:, :, ts(tc5, 512)]
                    )
                if first_rep:
                    nc.sync.dma_start(wo[:], wo_d[:])

                # ACT exp-table preload (overlaps phase A)
                wa = rpool.tile([128, 1], dt.float32, tag="warm")
                nc.scalar.activation(wa[:], bq[:, 0:1], AF.Exp)

                # --- QK projection pieces: one col-chunk accumulation burst
                qk_ps = {}

                def qk_mm(p, burst, dc):
                    """One matmul of pair-p's Q/K projection. K bursts first
                    (cc=4+p over 4 t-chunks), then Q bursts."""
                    cc = (4 + p) if burst < 4 else p
                    tc5 = burst % 4
                    if dc == 0:
                        qk_ps[p] = ps_qk.tile(
                            [128, 512], dt.float32, tag="ps_qk", name="qkps"
                        )
                    nc.tensor.matmul(
                        qk_ps[p][:],
                        wqk[:, dc, ts(cc, 128)],
                        xf[:, dc, ts(tc5, 512)],
                        start=(dc == 0),
                        stop=(dc == DCH - 1),
                    )
                    if dc == DCH - 1:
                        if cc < 4:
                            nc.vector.tensor_scalar_add(
                                qt[:, cc, ts(tc5, 512)], qk_ps[p][:],
                                bq[:, cc : cc + 1],
                            )
                        else:
                            nc.vector.tensor_copy(
                                kt[:, cc - 4, ts(tc5, 512)], qk_ps[p][:]
                            )

                # ---- phase A: V (all heads) + QK of pair 0 ----
                for b in range(4):  # K bursts of pair 0 first
                    for dc in range(DCH):
                        qk_mm(0, b, dc)
                for t8 in range(KCH):  # V: out [t128, 8 heads x 64]
                    vps = ps_qk.tile([128, 512], dt.float32, tag="ps_qk",
                                     name="vps")
                    for dc in range(DCH):
                        nc.tensor.matmul(
                            vps[:],
                            xf[:, dc, ts(t8, 128)],
                            wv[:, dc, :],
                            start=(dc == 0),
                            stop=(dc == DCH - 1),
                        )
                    nc.vector.tensor_copy(
                        vva[:, t8, :, 0:64],
                        vps[:].rearrange("p (h d) -> p h d", h=HPC),
                    )
                for b in range(4, 8):  # Q bursts of pair 0
                    for dc in range(DCH):
                        qk_mm(0, b, dc)

                # ---- phase B: attention; pair p+1's QK interleaved ----
                for pair in range(NPAIR):
                    slot = 0
                    for qc in range(4):  # 512-wide q chunks
                        avs = [
                            ps_av.tile([128, 512], dt.float32, tag="ps_av",
                                       name=f"av{_hh}")
                            for _hh in range(2)
                        ]
                        for kc in range(KCH):
                            s = ps_s.tile([128, 1024], dt.float32, tag="ps_s")
                            for hh in range(2):
                                base = 64 * hh
                                nc.tensor.matmul(
                                    s[:, ts(hh, 512)],
                                    kt[base : base + 64, pair, ts(kc, 128)],
                                    qt[base : base + 64, pair, ts(qc, 512)],
                                    start=True,
                                    stop=True,
                                )
                            pt = ptpool.tile([128, 1024], dt.bfloat16,
                                             tag="pt")
                            nc.scalar.activation(pt[:], s[:], AF.Exp)
                            for hh in range(2):
                                nc.tensor.matmul(
                                    avs[hh][:],
                                    vva[:, kc, pair * 2 + hh, :],
                                    pt[:, ts(hh, 512)],
                                    start=(kc == 0),
                                    stop=(kc == KCH - 1),
                                )
                            if pair < NPAIR - 1:
                                qk_mm(pair + 1, slot // 8, slot % 8)
                                slot += 1
                        for hh in range(2):
                            base = 64 * hh
                            # one copy frees the PSUM bank; recip+mul run on
                            # SBUF off the AV-accumulation critical path
                            avu = rpool.tile([128, 512], dt.float32, tag="avu",
                                             name="avu")
                            nc.vector.tensor_copy(avu[:], avs[hh][:])
                            rc = rpool.tile([64, 512], dt.float32, tag="recip")
                            nc.vector.reciprocal(rc[:], avu[64:128, :])
                            nc.vector.tensor_tensor(
                                attn[base : base + 64, pair, ts(qc, 512)],
                                avu[0:64, :],
                                rc[:],
                                mybir.AluOpType.mult,
                            )

                # ---- phase C: output projection (bias added on host) ----
                for t8 in range(T // 128):
                    ps = ps_s.tile([128, 1024], dt.float32, tag="ps_s")
                    for ncol in range(2):
                        for hc in range(NPAIR):
                            nc.tensor.matmul(
                                ps[:, ts(ncol, 512)],
                                attn[:, hc, ts(t8, 128)],
                                wo[:, hc, ts(ncol, 512)],
                                start=(hc == 0),
                                stop=(hc == NPAIR - 1),
                            )
                    ot = opool.tile([128, 1024], dt.float32, tag="outst")
                    nc.any.tensor_copy(ot[:], ps[:])
                    nc.sync.dma_start(out_d[ts(t8, 128), :], ot[:])

            for _rep in range(reps):
                emit_body(_rep == 0)

    nc.compile()
    return nc


def prep_in_maps(x, w_qkv, b_qkv, w_out, b_out):
    """Host-side shard prep. Returns (in_maps for cores 0..7, b_eff)."""
    x = np.asarray(x, dtype=np.float32)
    w_qkv = np.asarray(w_qkv, dtype=np.float32)
    b_qkv = np.asarray(b_qkv, dtype=np.float32)
    w_out = np.asarray(w_out, dtype=np.float32)
    b_out = np.asarray(b_out, dtype=np.float32)

    wq, wk, wv_full = w_qkv[:, 0:D], w_qkv[:, D : 2 * D], w_qkv[:, 2 * D : 3 * D]
    bq_full = b_qkv[0:D]
    bv = b_qkv[2 * D : 3 * D]
    b_eff = (bv @ w_out + b_out).astype(np.float32)

    in_maps = []
    for core in range(8):
        b, g = core // 2, core % 2
        gs = slice(g * 512, (g + 1) * 512)
        wqk_c = np.concatenate([wq[:, gs] * 0.125, wk[:, gs]], axis=1)  # [1024,1024]
        wqk_c = wqk_c.reshape(DCH, 128, 1024).transpose(1, 0, 2)
        wv_c = wv_full[:, gs].reshape(DCH, 128, 512).transpose(1, 0, 2)
        wo_c = w_out[gs, :].reshape(NPAIR, 128, 1024).transpose(1, 0, 2)
        bq_c = (bq_full[gs] * 0.125).reshape(NPAIR, 128).T
        xT_c = x[b].T.reshape(DCH, 128, T).transpose(1, 0, 2)
        in_maps.append(
            {
                "xT": np.ascontiguousarray(xT_c).astype(BF16),
                "wqk": np.ascontiguousarray(wqk_c).astype(BF16),
                "wv": np.ascontiguousarray(wv_c).astype(BF16),
                "wo": np.ascontiguousarray(wo_c).astype(BF16),
                "bq": np.ascontiguousarray(bq_c).astype(np.float32),
            }
        )
    return in_maps, b_eff


def gather_output(core_outs, b_eff):
    """core_outs: list of 8 [T, D] partials (core order b*2+g)."""
    out = np.empty((4, T, D), dtype=np.float32)
    for b in range(4):
        out[b] = core_outs[2 * b] + core_outs[2 * b + 1] + b_eff
    return out


_NC_CACHE = None


def _get_nc():
    global _NC_CACHE
    if _NC_CACHE is None:
        _NC_CACHE = build_nc(debug=False)
    return _NC_CACHE


def kernel(x, w_qkv, b_qkv, w_out, b_out):
    from concourse.bass_utils import run_bass_kernel_spmd

    in_maps, b_eff = prep_in_maps(x, w_qkv, b_qkv, w_out, b_out)
    nc = _get_nc()
    res = run_bass_kernel_spmd(nc, in_maps, core_ids=list(range(8)))
    return gather_output([r["out"] for r in res.results], b_eff)
